# revision 45
# baseline (speedup 1.0000x reference)
"""Epipolar attention kernel for Trainium2 (8 NeuronCores, batch-parallel).

Math notes (derived from the reference):
  - f_tar is dead code: the output only depends on f_src / K1 / K2 / R / t.
  - With x0=0, x1=W the distance field factorizes rank-3:
        d[b,i,j] = |px_i*alpha[b,j] + py_i*beta[b,j] + gamma[b,j]|
    where alpha = dy/L, beta = -dx/L, gamma = y0*dx/L, L = sqrt(dx^2+dy^2).
  - softmax_j(5*(d-0.1)) == softmax_j(5*d)           (shift invariance)
  - softmax_i(1 - p)     == softmax_i(-p), and p in (0,1] means exp(-p) needs
    no max subtraction.
The 3x3 SVD / inverse chain (B=16) plus the rank-3 coefficient prep is O(B*HW)
host work; all O(B*HW^2) work runs on the NeuronCores.
"""

import numpy as np
import ml_dtypes

import concourse.bass as bass
import concourse.bacc as bacc
import concourse.tile as tile
import concourse.mybir as mybir
from concourse.bass_utils import run_bass_kernel_spmd

B, C, H, W = 16, 512, 32, 32
HW = H * W          # 1024
NCORES = 8
BPC = B // NCORES   # batches per core
NT = HW // 128      # 128-row tiles per HW dim
F32 = mybir.dt.float32
BF16 = mybir.dt.bfloat16
AF = mybir.ActivationFunctionType
AX = mybir.AxisListType


# ---------------------------------------------------------------- host math
def _line_coeffs(K1, K2, R, t):
    """Float32 numpy mirror of the reference's per-batch line geometry.

    Returns Q (B, 3, HW) with rows [alpha, beta, gamma] and P (3, HW) with
    rows [px, py, 1].
    """
    K1 = np.asarray(K1, np.float32)
    K2 = np.asarray(K2, np.float32)
    R = np.asarray(R, np.float32)
    t = np.asarray(t, np.float32)

    z = np.zeros_like(t[:, 0])
    tx, ty, tz = t[:, 0], t[:, 1], t[:, 2]
    skew = np.stack(
        [
            np.stack([z, -tz, ty], axis=-1),
            np.stack([tz, z, -tx], axis=-1),
            np.stack([-ty, tx, z], axis=-1),
        ],
        axis=1,
    )
    E = skew @ R
    U, S, Vt = np.linalg.svd(E)
    S = S * np.array([1.0, 1.0, 0.0], dtype=S.dtype)
    E = U @ (S[:, :, None] * Vt)
    Fm = np.linalg.inv(np.swapaxes(K2, 1, 2)) @ E @ np.linalg.inv(K1)
    Fm = Fm.astype(np.float32)

    ix, iy = np.meshgrid(
        np.arange(H, dtype=np.float32), np.arange(W, dtype=np.float32), indexing="ij"
    )
    px = ix.reshape(-1)
    py = iy.reshape(-1)
    idx = np.stack([px, py, np.ones_like(px)], axis=0)  # (3, HW)

    lines = Fm @ idx[None]  # (B, 3, HW)
    a, b, c = lines[:, 0], lines[:, 1], lines[:, 2]
    x0 = np.zeros_like(a)
    y0 = -c / b
    x1 = np.full_like(a, float(W))
    y1 = -(c + a * float(W)) / b
    dx = x0 - x1
    dy = y0 - y1
    L = np.sqrt(dx * dx + dy * dy)

    alpha = dy / L
    beta = -dx / L
    gamma = (y0 * dx) / L
    Q = np.stack([alpha, beta, gamma], axis=1).astype(np.float32)  # (B, 3, HW)
    P = idx.astype(np.float32)
    return Q, P


# ---------------------------------------------------------------- device IR
def _build_nc():
    nc = bacc.Bacc("TRN2", target_bir_lowering=False, debug=False)

    pmat_d = nc.dram_tensor("pmat", [3, HW], BF16, kind="ExternalInput")
    # hi/lo bf16 split of the fp32 line coefficients: S = P^T Qhi + P^T Qlo
    # (P is exact in bf16), giving fp32-grade S at bf16 matmul speed.
    qmat_d = nc.dram_tensor("qmat", [BPC, 3, 2, HW], BF16, kind="ExternalInput")
    fsrc_d = nc.dram_tensor("fsrc", [BPC, HW, C], BF16, kind="ExternalInput")
    ident_d = nc.dram_tensor("ident", [128, 128], BF16, kind="ExternalInput")
    out_d = nc.dram_tensor("out", [BPC, HW, C], F32, kind="ExternalOutput")

    with tile.TileContext(nc) as tc:
        with (
            tc.tile_pool(name="const", bufs=1) as const,
            tc.tile_pool(name="q", bufs=2) as qpool,
            tc.tile_pool(name="f", bufs=2) as fpool,
            tc.tile_pool(name="z", bufs=4) as zpool,
            tc.tile_pool(name="e", bufs=2) as epool,
            tc.tile_pool(name="dg", bufs=2) as dgpool,
            tc.tile_pool(name="e2", bufs=2) as e2pool,
            tc.tile_pool(name="stat", bufs=2) as stat,
            tc.tile_pool(name="o", bufs=3) as opool,
            tc.tile_pool(name="sps", bufs=2, space="PSUM") as spspool,
            tc.tile_pool(name="ps", bufs=2, space="PSUM") as pspool,
        ):
            pm = const.tile([3, HW], BF16)
            nc.sync.dma_start(pm[:], pmat_d[:])
            idn = const.tile([128, 128], BF16)
            nc.sync.dma_start(idn[:], ident_d[:])

            st = [dict() for _ in range(BPC)]

            def load(b):
                s = st[b]
                s["q"] = qpool.tile([3, 2, HW], BF16, tag="q", name="q")
                nc.sync.dma_start(s["q"][:], qmat_d[b])
                s["fa"] = fpool.tile([128, NT, C], BF16, tag="fa", name="fa")
                for tj in range(NT):
                    nc.sync.dma_start(
                        s["fa"][:, tj, :], fsrc_d[b, tj * 128 : (tj + 1) * 128, :]
                    )
                s["ea"] = epool.tile([128, NT, HW], BF16, tag="ea", name="ea")
                s["ms"] = stat.tile([128, NT], F32, tag="ms", name="ms")
                s["s1"] = stat.tile([128, NT], F32, tag="s1", name="s1")
                s["r1"] = stat.tile([128, NT], F32, tag="r1", name="r1")
                s["dga"] = dgpool.tile([128, NT, 128], BF16, tag="dga", name="dga")
                s["e2"] = e2pool.tile([128, NT, HW], BF16, tag="e2", name="e2")
                s["s2"] = stat.tile([128, NT], F32, tag="s2", name="s2")
                s["r2"] = stat.tile([128, NT], F32, tag="r2", name="r2")

            def stage1(b, ti):
                # S = P^T Q (hi+lo), z = |5S|, row max, e = exp(z-m), 1/s1,
                # dg = diag(1/s1). p = e/s1 is folded into the PE transposes.
                s = st[b]
                sp = spspool.tile([128, HW], F32, tag="sp")
                for nh in range(2):
                    for hl in range(2):
                        nc.tensor.matmul(
                            sp[:, nh * 512 : (nh + 1) * 512],
                            pm[:, ti * 128 : (ti + 1) * 128],
                            s["q"][:, hl, nh * 512 : (nh + 1) * 512],
                            start=(hl == 0),
                            stop=(hl == 1),
                        )
                zt = zpool.tile([128, HW], F32)
                nc.scalar.activation(zt[:], sp[:], AF.Abs, scale=5.0)
                nc.vector.reduce_max(
                    s["ms"][:, ti : ti + 1], zt[:], axis=AX.X, negate=True
                )
                nc.scalar.activation(
                    s["ea"][:, ti, :],
                    zt[:],
                    AF.Exp,
                    bias=s["ms"][:, ti : ti + 1],
                    accum_out=s["s1"][:, ti : ti + 1],
                )
                nc.vector.reciprocal(
                    s["r1"][:, ti : ti + 1], s["s1"][:, ti : ti + 1]
                )
                nc.vector.tensor_scalar_mul(
                    s["dga"][:, ti, :], idn[:], s["r1"][:, ti : ti + 1]
                )

            def stage2(b, tj):
                # "transpose" via real matmul: PT[j,i'] = sum_i e[i,j]*dg[i,i']
                # = e[i',j]/s1[i'];  E2 = exp(-p) with column sums; fold 1/s2
                # into the f rows.
                s = st[b]
                tp = pspool.tile([128, HW], F32, tag="ps")
                # alternate PSUM banks between consecutive writes so the
                # bank-overlap tracker doesn't serialize back-to-back matmuls
                for ti in (0, 4, 1, 5, 2, 6, 3, 7):
                    nc.tensor.matmul(
                        tp[:, ti * 128 : (ti + 1) * 128],
                        s["ea"][:, ti, tj * 128 : (tj + 1) * 128],
                        s["dga"][:, ti, :],
                        start=True,
                        stop=True,
                    )
                nc.scalar.activation(
                    s["e2"][:, tj, :],
                    tp[:],
                    AF.Exp,
                    scale=-1.0,
                    accum_out=s["s2"][:, tj : tj + 1],
                )
                nc.vector.reciprocal(
                    s["r2"][:, tj : tj + 1], s["s2"][:, tj : tj + 1]
                )
                nc.vector.tensor_scalar_mul(
                    s["fa"][:, tj, :], s["fa"][:, tj, :], s["r2"][:, tj : tj + 1]
                )

            def stage3(b, tg):
                # GEMM: out[i, c] = sum_j exp(-p)[j,i] * fw[j, c]
                # Two i-tiles per 2-bank PSUM slot; one evict + DMA per pair.
                s = st[b]
                # GEMM PSUM tiles live in the stage-1 pool: its slots are free
                # during the GEMM phases, while the tp pool is still cycling
                # the other batch's transposes.
                op_ = spspool.tile([128, 2, C], F32, tag="sp")
                for half in range(2):
                    ti = 2 * tg + half
                    for tj in range(NT):
                        nc.tensor.matmul(
                            op_[:, half, :],
                            s["e2"][:, tj, ti * 128 : (ti + 1) * 128],
                            s["fa"][:, tj, :],
                            start=(tj == 0),
                            stop=(tj == NT - 1),
                        )
                ost = opool.tile([128, 2, C], F32)
                if b == BPC - 1:
                    # tail phase: ACT is idle during the last batch's GEMM
                    nc.scalar.copy(ost[:], op_[:])
                else:
                    nc.vector.tensor_copy(ost[:], op_[:])
                nc.sync.dma_start(
                    out_d[b, tg * 256 : (tg + 1) * 256, :].rearrange(
                        "(t p) c -> p t c", p=128
                    ),
                    ost[:],
                )

            # software-pipelined emission across the two batches so every
            # phase has dense work for both PE and ACT:
            #   s1(0) | s2(0)+s1(1) | s3(0)+s2(1) | s3(1)
            load(0)
            for ti in range(NT):
                stage1(0, ti)
            load(1)
            for k in range(NT):
                stage2(0, k)
                stage1(1, k)
            for k in range(NT):
                if k % 2 == 0:
                    stage3(0, k // 2)
                stage2(1, k)
            for tg in range(NT // 2):
                stage3(1, tg)
    nc.compile()
    return nc


_NC = None


def _get_nc():
    global _NC
    if _NC is None:
        _NC = _build_nc()
    return _NC


# ---------------------------------------------------------------- execution
def _run(inputs, trace=False):
    f_src = np.asarray(inputs["f_src"], np.float32)
    Q, P = _line_coeffs(inputs["K1"], inputs["K2"], inputs["R"], inputs["t"])

    fsrcT = np.ascontiguousarray(
        f_src.reshape(B, C, HW).transpose(0, 2, 1)
    ).astype(ml_dtypes.bfloat16)
    ident = np.eye(128, dtype=np.float32).astype(ml_dtypes.bfloat16)

    q_hi = Q.astype(ml_dtypes.bfloat16)
    q_lo = (Q - q_hi.astype(np.float32)).astype(ml_dtypes.bfloat16)
    q_hl = np.stack([q_hi, q_lo], axis=2)  # (B, 3, 2, HW) bf16
    p_bf = P.astype(ml_dtypes.bfloat16)  # exact: integers <= 32 and 1.0

    in_maps = []
    for core in range(NCORES):
        lo = core * BPC
        hi = lo + BPC
        in_maps.append(
            {
                "pmat": p_bf,
                "qmat": np.ascontiguousarray(q_hl[lo:hi]),
                "fsrc": np.ascontiguousarray(fsrcT[lo:hi]),
                "ident": ident,
            }
        )

    nc = _get_nc()
    res = run_bass_kernel_spmd(nc, in_maps, list(range(NCORES)), trace=trace)
    out_flat = np.concatenate(
        [res.results[i]["out"] for i in range(NCORES)], axis=0
    )  # (B, HW, C)
    out = np.ascontiguousarray(out_flat).reshape(B, C, H, W)
    return out, res


def kernel(**inputs):
    out, _ = _run(inputs, trace=False)
    return out
